# revision 13
# baseline (speedup 1.0000x reference)
"""Trainium2 Bass kernel for single-head causal attention.

Problem: B=4, T=4096, C=1024, HD=64 (fp32 inputs).
  q/k/v = x @ W{q,k,v};  scores = q k^T / sqrt(64), causal mask, softmax;
  out = attn @ v.

Sharding (8 cores, SPMD-uniform program):
  core = 2*batch + parity.  The two cores of a batch split the KEY axis into
  interleaved 256-column blocks (even blocks -> parity 0, odd -> parity 1).
  Each core computes, for ALL 4096 queries of its batch, the partial softmax
  numerator (sum_s exp(s_qs) v_s) and denominator (sum_s exp(s_qs)) over its
  own key blocks.  The host sums the two partials and divides.

v3 layout (from v2 trace analysis):
  - No PE warm-up junk matmuls: the first kv projection warms the HAM; the
    junk stream in v2 serialized on PSUM buffers and delayed real work ~4us.
  - Weights packed into ONE dram tensor wall=[Wk|Wv|Wq|Wq] (single DMA).
  - xt input DMA: chunk 0 split in two 512KB halves (fast pipeline start),
    chunks 1..7 as 1MB transfers (fewer Sync-queue DMA instructions, better
    per-transfer efficiency).  mask + all output DMAs go through the idle
    GpSimd SWDGE queue so they never delay the input stream.
  - PSUM budget: scores 2x[128,1024] (4 banks) + proj 2x[128,512] (2) +
    output 2x[65,512] (2) = 8 banks (slots are bank-granular).
  - Per chunk the q projection runs BEFORE kv (q gates the next group's
    scores; k/v only gate the group's diagonal, which is ordered LAST in
    jseq), and chunk g+2's projections are emitted INSIDE group g's
    attention stream so the Tile scheduler can fill PE gaps without
    stalling the Scalar (exp) queue at group boundaries.

  Scores are computed transposed (S^T[key, query]) so the PV contraction has
  keys on partitions; softmax max-subtraction is skipped (scores ~ N(0,1),
  exp can't overflow) and the denominator comes from a ones-column appended
  to V (output row 64).  Scores matmuls have K=64 contraction; two key tiles
  are row-packed into the 128x128 PE array and run concurrently.
"""

import os
import sys

import numpy as np

for _p in ("/opt/trn_rl_repo", "/root/.axon_site/_ro/trn_rl_repo"):
    if _p not in sys.path and os.path.isdir(_p):
        sys.path.append(_p)

import ml_dtypes  # noqa: E402

BF16 = ml_dtypes.bfloat16

B, T, C, HD = 4, 4096, 1024, 64
NCORES = 8
NG = 8          # query groups of 512 per batch
GQ = 512        # queries per group
KB = 256        # key block (one pair of 128-key tiles)
NKB = T // KB   # 16 global key blocks, 8 per core
CCH = C // 128  # 8 contraction chunks
SB = 512        # sub-block: 128 keys x 512 queries of scores
TPB = 2         # sub-blocks per exp tile (PSUM slots are bank-granular:
                # scores 2x2 banks + proj 2 + out 2 = 8 banks exactly)

_cache = {}


def _build_nc():
    import concourse.bass as bass
    import concourse.mybir as mybir
    import concourse.tile as tile
    from concourse import bacc
    from concourse.bass import ts

    fp32 = mybir.dt.float32
    bf16 = mybir.dt.bfloat16

    nc = bacc.Bacc("TRN2", target_bir_lowering=False, debug=False)

    xT = nc.dram_tensor("xT", [C, T], bf16, kind="ExternalInput")
    wall = nc.dram_tensor("wall", [C, 256], bf16, kind="ExternalInput")  # [Wk|Wv|Wq|Wq]
    maskd = nc.dram_tensor("maskd", [128, 1024], bf16, kind="ExternalInput")
    out_d = nc.dram_tensor("out", [HD + 1, T], fp32, kind="ExternalOutput")

    xT_v = xT[:, :].rearrange("(c p) t -> p c t", p=128)      # [128, 8, T]
    wall_v = wall[:, :].rearrange("(c p) m -> p c m", p=128)  # [128, 8, 256]

    from contextlib import ExitStack

    with tile.TileContext(nc) as tc, ExitStack() as ctx:
        singles = ctx.enter_context(tc.tile_pool(name="singles", bufs=1))
        ps_pj = ctx.enter_context(tc.tile_pool(name="ps_pj", bufs=2, space="PSUM"))
        ps_s = ctx.enter_context(tc.tile_pool(name="ps_s", bufs=2, space="PSUM"))
        ps_o = ctx.enter_context(tc.tile_pool(name="ps_o", bufs=2, space="PSUM"))
        pt_pool = ctx.enter_context(tc.tile_pool(name="pt", bufs=3))
        oe_pool = ctx.enter_context(tc.tile_pool(name="oe", bufs=2))

        # ---- persistent SBUF ----
        xt_sb = singles.tile([128, CCH, T], bf16, tag="xt")           # 64KB/part
        wall_sb = singles.tile([128, CCH, 256], bf16, tag="wall")
        kt_sb = singles.tile([128, T // 2], bf16, tag="kt")           # dup halves
        vt_sb = singles.tile([128, T // 2], bf16, tag="vt")           # rows 64:128
        qt_sb = singles.tile([128, T], bf16, tag="qt")                # dup halves
        vaug_sb = singles.tile([128, T // 2 // 128, HD + 1], bf16, tag="vaug")
        mask_sb = singles.tile([128, 1024], bf16, tag="mask")
        ident_sb = singles.tile([128, 64], bf16, tag="ident")

        # ---- input DMAs: weights first (small), then chunk 0 as two 512KB
        # halves so the first kv projection can start ~2.5us into the DMA
        # stream, then chunks 1..7 as 1MB transfers.  mask goes via the
        # (otherwise idle) gpsimd SWDGE queue, off the critical input path.
        nc.sync.dma_start(out=wall_sb[:, :, :], in_=wall_v[:, :, :])
        for hc in range(6):  # chunks 0..2 in 512KB halves for a fast ramp
            nc.sync.dma_start(out=xt_sb[:, :, ts(hc, 256)], in_=xT_v[:, :, ts(hc, 256)])
        nc.gpsimd.dma_start(out=mask_sb[:, :], in_=maskd[:, :])
        for c in range(3, NG):
            nc.sync.dma_start(out=xt_sb[:, :, ts(c, 512)], in_=xT_v[:, :, ts(c, 512)])

        # identity (rows 64:128) for PE transpose of V^T tiles
        nc.vector.memset(ident_sb[:, :], 0.0)
        nc.gpsimd.affine_select(
            out=ident_sb[:, :], in_=ident_sb[:, :],
            compare_op=mybir.AluOpType.not_equal, fill=1.0,
            base=-64, pattern=[[-1, 64]], channel_multiplier=1,
        )
        # only the denominator ones-column needs initializing; cols 0:HD are
        # fully written by the V-transpose copies
        nc.vector.memset(vaug_sb[:, :, HD:HD + 1], 1.0)

        def proj_kv(c):
            # kv projection over the own 256 columns (first half of chunk c)
            ps = ps_pj.tile([128, 512], fp32, tag="pj")
            for ch in range(CCH):
                nc.tensor.matmul(
                    ps[:, 0:256], lhsT=wall_sb[:, ch, 0:128],
                    rhs=xt_sb[:, ch, 512 * c: 512 * c + 256],
                    start=(ch == 0), stop=(ch == CCH - 1),
                )
            nc.vector.tensor_copy(out=kt_sb[0:64, ts(c, 256)], in_=ps[0:64, 0:256])
            nc.vector.tensor_copy(out=kt_sb[64:128, ts(c, 256)], in_=ps[0:64, 0:256])
            nc.vector.tensor_copy(out=vt_sb[64:128, ts(c, 256)], in_=ps[64:128, 0:256])
            # V^T -> V (PE transpose), rows of vaug get the ones col from memset
            for h in range(2):
                pst = ps_pj.tile([128, 64], bf16, tag="pj")
                nc.tensor.transpose(
                    out=pst[:, :],
                    in_=vt_sb[64:128, 256 * c + 128 * h: 256 * c + 128 * h + 128],
                    identity=ident_sb[64:128, :],
                )
                nc.vector.tensor_copy(out=vaug_sb[:, 2 * c + h, 0:HD], in_=pst[:, :])

        def proj_q(c):
            # q projection over all 512 columns ([Wq|Wq]: dup comes for free)
            psq = ps_pj.tile([128, 512], fp32, tag="pj")
            for half in range(2):
                for ch in range(CCH):
                    nc.tensor.matmul(
                        psq[:, 256 * half: 256 * half + 256],
                        lhsT=wall_sb[:, ch, 128:256],
                        rhs=xt_sb[:, ch, 512 * c + 256 * half: 512 * c + 256 * half + 256],
                        start=(ch == 0), stop=(ch == CCH - 1),
                    )
            nc.vector.tensor_copy(out=qt_sb[:, ts(c, 512)], in_=psq[:, :])

        # sub-block order per group: natural [0..g], diagonal LAST — the
        # diagonal is the only consumer of chunk g's k/vaug, so ordering it
        # last maximizes the projection pipeline's slack; its mask-multiply
        # overlaps the next group's exp.
        jseq = {g: list(range(g + 1)) for g in range(NG)}

        po_t = {}

        def attn_tile(g, subs, first, last):
            """One exp tile: scores matmuls + exp + masks + PV for a list of
            (j, h) sub-blocks of group g.  first/last: group PV boundaries."""
            n = len(subs)
            pss = ps_s.tile([128, TPB * SB], fp32, tag="ss")
            for i, (j, h) in enumerate(subs):
                nc.tensor.matmul(
                    pss[:, ts(i, SB)],
                    lhsT=kt_sb[64 * h: 64 * h + 64, KB * j + 128 * h: KB * j + 128 * h + 128],
                    rhs=qt_sb[64 * h: 64 * h + 64, ts(g, GQ)],
                    start=True, stop=True,
                )
            pt = pt_pool.tile([128, TPB * SB], bf16, tag="pt")
            nc.scalar.activation(
                out=pt[:, 0: n * SB], in_=pss[:, 0: n * SB],
                func=mybir.ActivationFunctionType.Exp, scale=0.125,
            )
            for i, (j, h) in enumerate(subs):
                if j == g:  # diagonal: causal mask (parity-specific data)
                    nc.vector.tensor_mul(
                        pt[:, ts(i, SB)], pt[:, ts(i, SB)], mask_sb[:, ts(h, SB)])
            for i, (j, h) in enumerate(subs):
                nc.tensor.matmul(
                    po_t[g][:, :],
                    lhsT=vaug_sb[:, 2 * j + h, :],
                    rhs=pt[:, ts(i, SB)],
                    start=(first and i == 0), stop=(last and i == n - 1),
                )

        def attn_evac(g):
            oe = oe_pool.tile([HD + 1, 512], fp32, tag="oe")
            nc.vector.tensor_copy(out=oe[:, :], in_=po_t[g][:, :])
            nc.gpsimd.dma_start(out=out_d[:, ts(g, 512)], in_=oe[:, :])

        # schedule: projections one chunk ahead of attention, emitted inside
        # the group's attention stream; the Tile scheduler's static cost
        # model does the fine-grained interleaving.
        proj_q(0)
        proj_kv(0)
        proj_q(1)
        proj_kv(1)
        for g in range(NG):
            subs = [(j, h) for j in jseq[g] for h in range(2)]
            tiles = [subs[i:i + TPB] for i in range(0, len(subs), TPB)]
            po_t[g] = ps_o.tile([HD + 1, 512], fp32, tag="po", name=f"po{g}")
            for t_idx, tsubs in enumerate(tiles):
                attn_tile(g, tsubs, first=(t_idx == 0), last=(t_idx == len(tiles) - 1))
                if g + 2 < NG:
                    if t_idx == 0:
                        proj_q(g + 2)
                    if t_idx == min(1, len(tiles) - 1):
                        proj_kv(g + 2)
            attn_evac(g)

    nc.compile()
    return nc


def _get_nc():
    if "nc" not in _cache:
        _cache["nc"] = _build_nc()
    return _cache["nc"]


def _perm(parity: int) -> np.ndarray:
    # chunk-local order: chunk c = [global block 2c+parity | block 2c+1-parity]
    blocks = np.arange(NKB).reshape(NG, 2)           # [[0,1],[2,3],...]
    if parity == 1:
        blocks = blocks[:, ::-1]
    return (blocks.reshape(-1)[:, None] * KB + np.arange(KB)[None, :]).ravel()


def _mask(parity: int) -> np.ndarray:
    r = np.arange(128)[:, None]
    j = np.arange(KB)[None, :]
    tri0 = (r <= j).astype(np.float32)            # key tile h=0 vs own block
    tri1 = (128 + r <= j).astype(np.float32)      # key tile h=1
    second = np.ones((128, KB), np.float32) if parity == 0 else np.zeros(
        (128, KB), np.float32)
    m = np.concatenate([tri0, second, tri1, second], axis=1)  # [128, 1024]
    return m.astype(BF16)


def _in_maps(x, Wq, Wk, Wv):
    wall = np.concatenate([Wk, Wv, Wq, Wq], axis=1).astype(BF16)
    masks = [_mask(0), _mask(1)]
    perm1 = _perm(1)
    in_maps = []
    for core in range(NCORES):
        b, par = core // 2, core % 2
        xTb = np.ascontiguousarray(x[b].T).astype(BF16)
        xT = xTb if par == 0 else np.ascontiguousarray(xTb[:, perm1])
        in_maps.append({"xT": xT, "wall": wall, "maskd": masks[par]})
    return in_maps


def _combine(outs):
    """outs: 8 arrays [65, T] fp32 -> full [B, T, HD] fp32."""
    full = np.empty((B, T, HD), np.float32)
    for b in range(B):
        oe = outs[2 * b]
        oo = outs[2 * b + 1].reshape(HD + 1, NG, 2, KB)[:, :, ::-1, :].reshape(
            HD + 1, T)
        num = oe[0:HD] + oo[0:HD]
        den = oe[HD] + oo[HD]
        full[b] = (num / den).T
    return full


def run(x, Wq, Wk, Wv, trace=False):
    from concourse.bass_utils import run_bass_kernel_spmd

    nc = _get_nc()
    in_maps = _in_maps(x, Wq, Wk, Wv)
    res = run_bass_kernel_spmd(
        nc, in_maps, core_ids=list(range(NCORES)), trace=trace,
    )
    outs = [r["out"] for r in res.results]
    return _combine(outs), res


def kernel(x, Wq, Wk, Wv, padding_mask=None, **_ignored):
    out, _ = run(np.asarray(x, np.float32), np.asarray(Wq, np.float32),
                 np.asarray(Wk, np.float32), np.asarray(Wv, np.float32))
    return out
